# revision 19
# baseline (speedup 1.0000x reference)
"""CrossBandAttention Trainium2 kernel.

Math: 3 bands [B,C,H,W] -> per-band 1x1 conv (C->E) -> MHA over the 3-band
sequence per pixel -> out-proj -> per-band 1x1 conv (E->C) -> gated residual.

All linear stages are merged on the host into tiny per-band matrices acting on
the raw 9 input channels (3 bands x 3 chans), so the device kernel is, per
pixel: a handful of [9 -> 72] matmuls, the 3x3x8 score bilinear form, exp,
softmax-normalize, attn-weighted sum, residual. Layout is feature-major:
SBUF/PSUM tiles [rows, FD-pixels], pixels streamed in FD=512 chunks.

Row orderings:
  aug rows   r = 3j + c            (band j, channel c)       [9]
  t/y rows   m = i*24 + h*3 + a    (query band i, head h, a)  [72]
  e/z/T rows m = c*24 + i*8 + h    (payload chan c, i, h)     [72]
  den rows   m = c*24 + i*8 + h    (c-replicated)             [72]
  out rows   m = 3i + c                                        [9]
"""

import math

import numpy as np

B, C, H, W = 4, 3, 256, 256
E, HEADS, HD = 64, 8, 8
NCORES = 8
PIX = B * H * W // NCORES  # 32768 pixels per core
FD = 512                   # pixels per chunk
NCHUNK = PIX // FD

IH = 24   # (i, h) pairs
ROWS = 72


def _merged_weights(Wp, bp, in_proj_w, in_proj_b, out_proj_w, out_proj_b,
                    Wo, bo, gates):
    """Fold every linear stage into small fp32 matrices. float64 internally."""
    f8 = np.float64
    Wp, bp = Wp.astype(f8), bp.astype(f8)
    ipw, ipb = in_proj_w.astype(f8), in_proj_b.astype(f8)
    opw, opb = out_proj_w.astype(f8), out_proj_b.astype(f8)
    Wo, bo = Wo.astype(f8), bo.astype(f8)
    g = gates.astype(f8)
    w = np.exp(g - g.max())
    w /= w.sum()

    # per-band merged q/k/v from (3 chans + const): [3][64, 4]
    QA = np.zeros((3, E, 4))
    KA = np.zeros((3, E, 4))
    VA = np.zeros((3, E, 4))
    for j in range(3):
        for blk, M in ((0, QA), (1, KA), (2, VA)):
            r0 = blk * E
            M[j, :, :3] = ipw[r0:r0 + E] @ Wp[j]
            M[j, :, 3] = ipw[r0:r0 + E] @ bp[j] + ipb[r0:r0 + E]

    # score bilinear forms: S[i,j,h] in R^{4x4}
    S = np.zeros((3, 3, HEADS, 4, 4))
    for i in range(3):
        for j in range(3):
            for h in range(HEADS):
                qb = QA[i, h * HD:(h + 1) * HD]      # [8, 4]
                kb = KA[j, h * HD:(h + 1) * HD]
                S[i, j, h] = qb.T @ kb / math.sqrt(HD)

    # output-side merge: M_ih [3, 8] maps head-h v-components to band-i chans
    WoP = np.einsum('ice,ef->icf', Wo, opw)          # [3, C, E]
    Mih = np.zeros((3, HEADS, 3, HD))
    for i in range(3):
        for h in range(HEADS):
            Mih[i, h] = w[i] * WoP[i][:, h * HD:(h + 1) * HD]
    b_eff = (np.einsum('ice,e->ic', Wo, opb) + bo) * w[:, None]   # [3, C]

    def em(c, i, h):  # e/z row index
        return c * 24 + i * 8 + h

    # ---- pair-product form for the score bilinears ----
    # m2[p*9 + a*3 + b] = band_{P1(p)}[a] * band_{P2(p)}[b] over 6 band pairs
    PAIRS = [(0, 1), (0, 2), (1, 2), (0, 0), (1, 1), (2, 2)]
    M2 = 54
    RA = np.zeros((9, M2))
    RB = np.zeros((9, M2))
    for p, (p1, p2) in enumerate(PAIRS):
        for a in range(3):
            for b in range(3):
                RA[3 * p1 + a, p * 9 + a * 3 + b] = 1.0
                RB[3 * p2 + b, p * 9 + a * 3 + b] = 1.0

    WS = np.zeros((3, M2, ROWS))           # m2 rows -> e rows, per source band j
    for j in range(3):
        for c in range(3):
            for i in range(3):
                for h in range(HEADS):
                    m = em(c, i, h)
                    if i == j:
                        p = PAIRS.index((i, i))
                        for a in range(3):
                            for b in range(3):
                                WS[j, p * 9 + a * 3 + b, m] += S[i, j, h][a, b]
                    else:
                        p = PAIRS.index((min(i, j), max(i, j)))
                        p1, p2 = PAIRS[p]
                        for a in range(3):
                            for b in range(3):
                                if (p1, p2) == (i, j):
                                    WS[j, p * 9 + a * 3 + b, m] += S[i, j, h][a, b]
                                else:
                                    WS[j, p * 9 + a * 3 + b, m] += S[i, j, h][b, a]

    VAl = np.stack([VA[j][:, :3] for j in range(3)])       # [3, 64, 3]
    vc = np.stack([VA[j][:, 3] for j in range(3)])         # [3, 64]
    Wz = np.zeros((3, 9, ROWS))
    zc = np.zeros((3, ROWS))
    for j in range(3):
        for c in range(3):
            for i in range(3):
                for h in range(HEADS):
                    m = em(c, i, h)
                    for b in range(3):
                        Wz[j, 3 * j + b, m] = Mih[i, h][c] @ VAl[j, h * HD:(h + 1) * HD, b]
                    zc[j, m] = Mih[i, h][c] @ vc[j, h * HD:(h + 1) * HD]

    Lin = np.zeros((3, 9, ROWS))
    EB = np.zeros((ROWS, 3))
    for j in range(3):
        for c in range(3):
            for i in range(3):
                for h in range(HEADS):
                    m = em(c, i, h)
                    for a in range(3):
                        Lin[j, 3 * i + a, m] += S[i, j, h][a, 3]
                    for b in range(3):
                        Lin[j, 3 * j + b, m] += S[i, j, h][3, b]
                    EB[m, j] = S[i, j, h][3, 3]

    Ires = np.eye(9)

    # ---- W72 blocks ----
    I72 = np.eye(ROWS)
    Wh = np.zeros((ROWS, 9))
    for c in range(3):
        for i in range(3):
            for h in range(HEADS):
                Wh[em(c, i, h), 3 * i + c] = 1.0

    Ibc = np.zeros((IH, ROWS))             # den c-replication [24 -> 72]
    for c in range(3):
        for i in range(3):
            for h in range(HEADS):
                Ibc[i * 8 + h, em(c, i, h)] = 1.0

    f4 = np.float32
    W9 = np.concatenate([RA, RB] + [Wz[j] for j in range(3)]
                        + [Lin[j] for j in range(3)] + [Ires], axis=1)
    W72 = np.concatenate([I72, Wh], axis=1)
    W54 = np.concatenate([WS[j] for j in range(3)], axis=1)
    return {
        'W9': W9.astype(f4),               # [9, 549]
        'W72': W72.astype(f4),             # [72, 81]
        'W54': W54.astype(f4),             # [54, 216]
        'W24': Ibc.astype(f4),             # [24, 72]
        'EB': EB.astype(f4),               # [72, 3]
        'ZC': zc.T.astype(f4),             # [72, 3]
        'BIAS9': b_eff.reshape(9, 1).astype(f4),
    }


# column offsets inside W9 / W72 / W54
M2 = 54
W9_RA, W9_RB = 0, M2
W9_Z = [2 * M2 + j * ROWS for j in range(3)]
W9_LIN = [2 * M2 + 3 * ROWS + j * ROWS for j in range(3)]
W9_IRES = 2 * M2 + 6 * ROWS
W72_I72, W72_WH = 0, ROWS
W54_WS = [j * ROWS for j in range(3)]


def golden_core(xb, wts):
    """Numpy emulation of the device program for one core. xb: [3][3, pix]."""
    pix = xb[0].shape[1]
    aug = np.concatenate([xb[0], xb[1], xb[2]], axis=0).astype(np.float32)  # [9, pix]
    W9, W72, W54, W24 = wts['W9'], wts['W72'], wts['W54'], wts['W24']
    m2 = (W9[:, W9_RA:W9_RA + M2].T @ aug) * (W9[:, W9_RB:W9_RB + M2].T @ aug)
    es, ps = [], []
    for j in range(3):
        sc = W54[:, W54_WS[j]:W54_WS[j] + ROWS].T @ m2 \
            + W9[:, W9_LIN[j]:W9_LIN[j] + ROWS].T @ aug
        e = np.exp(sc + wts['EB'][:, j:j + 1])
        es.append(e)
    for j in range(3):
        z = W9[:, W9_Z[j]:W9_Z[j] + ROWS].T @ aug
        ps.append((z + wts['ZC'][:, j:j + 1]) * es[j])
    T = sum(W72[:, W72_I72:W72_I72 + ROWS].T @ p for p in ps)
    den = sum(W24.T @ e[:IH] for e in es)
    msb = T * (1.0 / den)
    out9 = W72[:, W72_WH:W72_WH + 9].T @ msb \
        + W9[:, W9_IRES:W9_IRES + 9].T @ aug + wts['BIAS9']
    return out9.astype(np.float32)  # [9, pix] rows 3i+c


def build_program(pix=PIX, fd=FD):
    import concourse.bacc as bacc
    import concourse.mybir as mybir
    from concourse.tile import TileContext

    f32 = mybir.dt.float32
    AF = mybir.ActivationFunctionType
    nc = bacc.Bacc("TRN2")

    xall = nc.dram_tensor("xall", [9, pix], f32, kind="ExternalInput").ap()
    w9_d = nc.dram_tensor("w9", [9, 549], f32, kind="ExternalInput").ap()
    w72_d = nc.dram_tensor("w72", [ROWS, 81], f32, kind="ExternalInput").ap()
    w54_d = nc.dram_tensor("w54", [M2, 216], f32, kind="ExternalInput").ap()
    w24_d = nc.dram_tensor("w24", [IH, ROWS], f32, kind="ExternalInput").ap()
    eb_d = nc.dram_tensor("eb", [ROWS, 3], f32, kind="ExternalInput").ap()
    zc_d = nc.dram_tensor("zc", [ROWS, 3], f32, kind="ExternalInput").ap()
    b9_d = nc.dram_tensor("b9", [9, 1], f32, kind="ExternalInput").ap()
    oall = nc.dram_tensor("oall", [9, pix], f32, kind="ExternalOutput").ap()

    with TileContext(nc) as tc:
        with (
            tc.tile_pool(name="const", bufs=1) as cp,
            tc.tile_pool(name="sb", bufs=3) as sb,
            tc.tile_pool(name="ps", bufs=2, space="PSUM") as pp,
        ):
            w9 = cp.tile([9, 549], f32)
            w72 = cp.tile([ROWS, 81], f32)
            w54 = cp.tile([M2, 216], f32)
            w24 = cp.tile([IH, ROWS], f32)
            eb = cp.tile([ROWS, 3], f32)
            zc = cp.tile([ROWS, 3], f32)
            b9 = cp.tile([9, 1], f32)
            nc.sync.dma_start(w9[:], w9_d)
            nc.sync.dma_start(w72[:], w72_d)
            nc.sync.dma_start(w54[:], w54_d)
            nc.sync.dma_start(w24[:], w24_d)
            nc.sync.dma_start(eb[:], eb_d)
            nc.sync.dma_start(zc[:], zc_d)
            nc.sync.dma_start(b9[:], b9_d)
            # Dummy PE consumers of each const so later matmuls never wait on
            # two DMA-HW queues at once (the LDWEIGHTS struct can hold only
            # one DGE sync-wait).
            warm = pp.tile([1, 8], f32, tag="A")
            for wt in (w9, w72, w54, w24):
                nc.tensor.matmul(warm[:], wt[:, 0:1], wt[:, 0:8],
                                 start=True, stop=True)

            for ci in range(pix // fd):
                o = ci * fd
                aug = sb.tile([9, fd], f32, tag="aug")
                nc.sync.dma_start(aug[:], xall[:, o:o + fd])

                ra_p = pp.tile([M2, fd], f32, tag="A")
                nc.tensor.matmul(ra_p[:], w9[:, W9_RA:W9_RA + M2], aug[:],
                                 start=True, stop=True)
                rb_p = pp.tile([M2, fd], f32, tag="y")
                nc.tensor.matmul(rb_p[:], w9[:, W9_RB:W9_RB + M2], aug[:],
                                 start=True, stop=True)
                rb_s = sb.tile([M2, fd], f32, tag="rb")
                nc.scalar.copy(rb_s[:], rb_p[:])
                m2_s = sb.tile([M2, fd], f32, tag="m2")
                nc.vector.tensor_mul(m2_s[:], ra_p[:], rb_s[:])

                es = []
                for j in range(3):
                    sc_p = pp.tile([ROWS, fd], f32, tag="sc")
                    nc.tensor.matmul(sc_p[:], w54[:, W54_WS[j]:W54_WS[j] + ROWS],
                                     m2_s[:], start=True, stop=False)
                    nc.tensor.matmul(sc_p[:], w9[:, W9_LIN[j]:W9_LIN[j] + ROWS],
                                     aug[:], start=False, stop=True)
                    e_j = sb.tile([ROWS, fd], f32, tag=f"e{j}")
                    nc.scalar.activation(e_j[:], sc_p[:], AF.Exp,
                                         bias=eb[:, j:j + 1])
                    es.append(e_j)

                ps_l = []
                for j in range(3):
                    z_p = pp.tile([ROWS, fd], f32, tag="z")
                    nc.tensor.matmul(z_p[:], w9[:, W9_Z[j]:W9_Z[j] + ROWS], aug[:],
                                     start=True, stop=True)
                    p_j = sb.tile([ROWS, fd], f32, tag=f"p{j}")
                    # p = (z + zc_j) * e   (zc_j is a per-partition scalar)
                    nc.vector.scalar_tensor_tensor(
                        p_j[:], z_p[:], zc[:, j:j + 1], es[j][:],
                        op0=mybir.AluOpType.add, op1=mybir.AluOpType.mult)
                    ps_l.append(p_j)

                T_p = pp.tile([ROWS, fd], f32, tag="A")
                for j in range(3):
                    nc.tensor.matmul(T_p[:], w72[:, W72_I72:W72_I72 + ROWS],
                                     ps_l[j][:], start=(j == 0), stop=(j == 2))
                den_p = pp.tile([ROWS, fd], f32, tag="sc")
                for j in range(3):
                    nc.tensor.matmul(den_p[:], w24[:], es[j][0:IH, :],
                                     start=(j == 0), stop=(j == 2))
                rcp = sb.tile([ROWS, fd], f32, tag="rcp")
                nc.vector.reciprocal(rcp[:], den_p[:])
                msb = sb.tile([ROWS, fd], f32, tag="msb")
                nc.vector.tensor_mul(msb[:], T_p[:], rcp[:])

                out9_p = pp.tile([9, fd], f32, tag="y")
                nc.tensor.matmul(out9_p[:], w72[:, W72_WH:W72_WH + 9], msb[:],
                                 start=True, stop=False)
                # (Ires: residual band add; bias added in the Identity copy)
                nc.tensor.matmul(out9_p[:], w9[:, W9_IRES:W9_IRES + 9], aug[:],
                                 start=False, stop=True)
                res = sb.tile([9, fd], f32, tag="res")
                nc.scalar.activation(res[:], out9_p[:], AF.Identity,
                                     bias=b9[:, 0:1])
                nc.sync.dma_start(oall[:, o:o + fd], res[:])
    nc.compile()
    return nc


def _shard_inputs(band0, band1, band2, wts):
    bands = [np.ascontiguousarray(b, dtype=np.float32)
             for b in (band0, band1, band2)]
    in_maps = []
    for k in range(NCORES):
        b, half = k // 2, k % 2
        m = dict(w9=wts['W9'], w72=wts['W72'], w54=wts['W54'], w24=wts['W24'],
                 eb=wts['EB'], zc=wts['ZC'], b9=wts['BIAS9'])
        m["xall"] = np.concatenate(
            [bands[j][b, :, half * 128:half * 128 + 128, :].reshape(3, PIX)
             for j in range(3)], axis=0)
        in_maps.append(m)
    return in_maps


def _unshard(results):
    full = [np.empty((B, C, H, W), np.float32) for _ in range(3)]
    for k in range(NCORES):
        b, half = k // 2, k % 2
        for j in range(3):
            full[j][b, :, half * 128:half * 128 + 128, :] = \
                results[k]["oall"][3 * j:3 * j + 3].reshape(3, 128, W)
    return tuple(full)


def kernel(band0, band1, band2, Wp, bp, in_proj_w, in_proj_b,
           out_proj_w, out_proj_b, Wo, bo, gates):
    from concourse.bass_utils import run_bass_kernel_spmd

    wts = _merged_weights(np.asarray(Wp), np.asarray(bp), np.asarray(in_proj_w),
                          np.asarray(in_proj_b), np.asarray(out_proj_w),
                          np.asarray(out_proj_b), np.asarray(Wo),
                          np.asarray(bo), np.asarray(gates))
    nc = build_program()
    in_maps = _shard_inputs(np.asarray(band0), np.asarray(band1),
                            np.asarray(band2), wts)
    r = run_bass_kernel_spmd(nc, in_maps, core_ids=list(range(NCORES)))
    return _unshard(r.results)


# revision 21
# speedup vs baseline: 169.5463x; 169.5463x over previous
"""CrossBandAttention Trainium2 kernel.

Math: 3 bands [B,C,H,W] -> per-band 1x1 conv (C->E) -> MHA over the 3-band
sequence per pixel -> out-proj -> per-band 1x1 conv (E->C) -> gated residual.

All linear stages are merged on the host into tiny per-band matrices acting on
the raw 9 input channels (3 bands x 3 chans), so the device kernel is, per
pixel: a handful of [9 -> 72] matmuls, the 3x3x8 score bilinear form, exp,
softmax-normalize, attn-weighted sum, residual. Layout is feature-major:
SBUF/PSUM tiles [rows, FD-pixels], pixels streamed in FD=512 chunks.

Row orderings:
  aug rows   r = 3j + c            (band j, channel c)       [9]
  t/y rows   m = i*24 + h*3 + a    (query band i, head h, a)  [72]
  e/z/T rows m = c*24 + i*8 + h    (payload chan c, i, h)     [72]
  den rows   m = c*24 + i*8 + h    (c-replicated)             [72]
  out rows   m = 3i + c                                        [9]
"""

import math

import numpy as np

B, C, H, W = 4, 3, 256, 256
E, HEADS, HD = 64, 8, 8
NCORES = 8
PIX = B * H * W // NCORES  # 32768 pixels per core
FD = 512                   # pixels per chunk
NCHUNK = PIX // FD

IH = 24   # (i, h) pairs
ROWS = 72


def _merged_weights(Wp, bp, in_proj_w, in_proj_b, out_proj_w, out_proj_b,
                    Wo, bo, gates):
    """Fold every linear stage into small fp32 matrices. float64 internally."""
    f8 = np.float64
    Wp, bp = Wp.astype(f8), bp.astype(f8)
    ipw, ipb = in_proj_w.astype(f8), in_proj_b.astype(f8)
    opw, opb = out_proj_w.astype(f8), out_proj_b.astype(f8)
    Wo, bo = Wo.astype(f8), bo.astype(f8)
    g = gates.astype(f8)
    w = np.exp(g - g.max())
    w /= w.sum()

    # per-band merged q/k/v from (3 chans + const): [3][64, 4]
    QA = np.zeros((3, E, 4))
    KA = np.zeros((3, E, 4))
    VA = np.zeros((3, E, 4))
    for j in range(3):
        for blk, M in ((0, QA), (1, KA), (2, VA)):
            r0 = blk * E
            M[j, :, :3] = ipw[r0:r0 + E] @ Wp[j]
            M[j, :, 3] = ipw[r0:r0 + E] @ bp[j] + ipb[r0:r0 + E]

    # score bilinear forms: S[i,j,h] in R^{4x4}
    S = np.zeros((3, 3, HEADS, 4, 4))
    for i in range(3):
        for j in range(3):
            for h in range(HEADS):
                qb = QA[i, h * HD:(h + 1) * HD]      # [8, 4]
                kb = KA[j, h * HD:(h + 1) * HD]
                S[i, j, h] = qb.T @ kb / math.sqrt(HD)

    # output-side merge: M_ih [3, 8] maps head-h v-components to band-i chans
    WoP = np.einsum('ice,ef->icf', Wo, opw)          # [3, C, E]
    Mih = np.zeros((3, HEADS, 3, HD))
    for i in range(3):
        for h in range(HEADS):
            Mih[i, h] = w[i] * WoP[i][:, h * HD:(h + 1) * HD]
    b_eff = (np.einsum('ice,e->ic', Wo, opb) + bo) * w[:, None]   # [3, C]

    def em(c, i, h):  # e/z row index
        return c * 24 + i * 8 + h

    # ---- pair-product form for the score bilinears ----
    # m2[p*9 + a*3 + b] = band_{P1(p)}[a] * band_{P2(p)}[b] over 6 band pairs
    PAIRS = [(0, 1), (0, 2), (1, 2), (0, 0), (1, 1), (2, 2)]
    M2 = 54
    RA = np.zeros((9, M2))
    RB = np.zeros((9, M2))
    for p, (p1, p2) in enumerate(PAIRS):
        for a in range(3):
            for b in range(3):
                RA[3 * p1 + a, p * 9 + a * 3 + b] = 1.0
                RB[3 * p2 + b, p * 9 + a * 3 + b] = 1.0

    WS = np.zeros((3, M2, ROWS))           # m2 rows -> e rows, per source band j
    for j in range(3):
        for c in range(3):
            for i in range(3):
                for h in range(HEADS):
                    m = em(c, i, h)
                    if i == j:
                        p = PAIRS.index((i, i))
                        for a in range(3):
                            for b in range(3):
                                WS[j, p * 9 + a * 3 + b, m] += S[i, j, h][a, b]
                    else:
                        p = PAIRS.index((min(i, j), max(i, j)))
                        p1, p2 = PAIRS[p]
                        for a in range(3):
                            for b in range(3):
                                if (p1, p2) == (i, j):
                                    WS[j, p * 9 + a * 3 + b, m] += S[i, j, h][a, b]
                                else:
                                    WS[j, p * 9 + a * 3 + b, m] += S[i, j, h][b, a]

    VAl = np.stack([VA[j][:, :3] for j in range(3)])       # [3, 64, 3]
    vc = np.stack([VA[j][:, 3] for j in range(3)])         # [3, 64]
    Wz = np.zeros((3, 9, ROWS))
    zc = np.zeros((3, ROWS))
    for j in range(3):
        for c in range(3):
            for i in range(3):
                for h in range(HEADS):
                    m = em(c, i, h)
                    for b in range(3):
                        Wz[j, 3 * j + b, m] = Mih[i, h][c] @ VAl[j, h * HD:(h + 1) * HD, b]
                    zc[j, m] = Mih[i, h][c] @ vc[j, h * HD:(h + 1) * HD]

    Lin = np.zeros((3, 9, ROWS))
    EB = np.zeros((ROWS, 3))
    for j in range(3):
        for c in range(3):
            for i in range(3):
                for h in range(HEADS):
                    m = em(c, i, h)
                    for a in range(3):
                        Lin[j, 3 * i + a, m] += S[i, j, h][a, 3]
                    for b in range(3):
                        Lin[j, 3 * j + b, m] += S[i, j, h][3, b]
                    EB[m, j] = S[i, j, h][3, 3]

    Ires = np.eye(9)

    # ---- W72 blocks ----
    I72 = np.eye(ROWS)
    Wh = np.zeros((ROWS, 9))
    for c in range(3):
        for i in range(3):
            for h in range(HEADS):
                Wh[em(c, i, h), 3 * i + c] = 1.0

    Ibc = np.zeros((IH, ROWS))             # den c-replication [24 -> 72]
    for c in range(3):
        for i in range(3):
            for h in range(HEADS):
                Ibc[i * 8 + h, em(c, i, h)] = 1.0

    f4 = np.float32
    W9 = np.concatenate([RA, RB] + [Wz[j] for j in range(3)]
                        + [Lin[j] for j in range(3)] + [Ires], axis=1)
    W72 = np.concatenate([I72, Wh], axis=1)
    W54 = np.concatenate([WS[j] for j in range(3)], axis=1)
    return {
        'W9': W9.astype(f4),               # [9, 549]
        'W72': W72.astype(f4),             # [72, 81]
        'W54': W54.astype(f4),             # [54, 216]
        'W24': Ibc.astype(f4),             # [24, 72]
        'EB': EB.astype(f4),               # [72, 3]
        'ZC': zc.T.astype(f4),             # [72, 3]
        'BIAS9': b_eff.reshape(9, 1).astype(f4),
    }


# column offsets inside W9 / W72 / W54
M2 = 54
W9_RA, W9_RB = 0, M2
W9_Z = [2 * M2 + j * ROWS for j in range(3)]
W9_LIN = [2 * M2 + 3 * ROWS + j * ROWS for j in range(3)]
W9_IRES = 2 * M2 + 6 * ROWS
W72_I72, W72_WH = 0, ROWS
W54_WS = [j * ROWS for j in range(3)]


def golden_core(xb, wts):
    """Numpy emulation of the device program for one core. xb: [3][3, pix]."""
    pix = xb[0].shape[1]
    aug = np.concatenate([xb[0], xb[1], xb[2]], axis=0).astype(np.float32)  # [9, pix]
    W9, W72, W54, W24 = wts['W9'], wts['W72'], wts['W54'], wts['W24']
    m2 = (W9[:, W9_RA:W9_RA + M2].T @ aug) * (W9[:, W9_RB:W9_RB + M2].T @ aug)
    es, ps = [], []
    for j in range(3):
        sc = W54[:, W54_WS[j]:W54_WS[j] + ROWS].T @ m2 \
            + W9[:, W9_LIN[j]:W9_LIN[j] + ROWS].T @ aug
        e = np.exp(sc + wts['EB'][:, j:j + 1])
        es.append(e)
    for j in range(3):
        z = W9[:, W9_Z[j]:W9_Z[j] + ROWS].T @ aug
        ps.append((z + wts['ZC'][:, j:j + 1]) * es[j])
    T = sum(W72[:, W72_I72:W72_I72 + ROWS].T @ p for p in ps)
    den = sum(W24.T @ e[:IH] for e in es)
    msb = T * (1.0 / den)
    out9 = W72[:, W72_WH:W72_WH + 9].T @ msb \
        + W9[:, W9_IRES:W9_IRES + 9].T @ aug + wts['BIAS9']
    return out9.astype(np.float32)  # [9, pix] rows 3i+c


def build_program(pix=PIX, fd=FD, reps=1):
    """reps>1 wraps the whole pixel loop in a device-side For_i so kernel
    time can be measured as the wall-clock delta between rep counts."""
    import concourse.bacc as bacc
    import concourse.mybir as mybir
    from concourse.tile import TileContext
    from contextlib import nullcontext

    f32 = mybir.dt.float32
    AF = mybir.ActivationFunctionType
    nc = bacc.Bacc("TRN2")

    xall = nc.dram_tensor("xall", [9, pix], f32, kind="ExternalInput").ap()
    w9_d = nc.dram_tensor("w9", [9, 549], f32, kind="ExternalInput").ap()
    w72_d = nc.dram_tensor("w72", [ROWS, 81], f32, kind="ExternalInput").ap()
    w54_d = nc.dram_tensor("w54", [M2, 216], f32, kind="ExternalInput").ap()
    w24_d = nc.dram_tensor("w24", [IH, ROWS], f32, kind="ExternalInput").ap()
    eb_d = nc.dram_tensor("eb", [ROWS, 3], f32, kind="ExternalInput").ap()
    zc_d = nc.dram_tensor("zc", [ROWS, 3], f32, kind="ExternalInput").ap()
    b9_d = nc.dram_tensor("b9", [9, 1], f32, kind="ExternalInput").ap()
    oall = nc.dram_tensor("oall", [9, pix], f32, kind="ExternalOutput").ap()

    with TileContext(nc) as tc:
        with (
            tc.tile_pool(name="const", bufs=1) as cp,
            tc.tile_pool(name="sb", bufs=3) as sb,
            tc.tile_pool(name="ps", bufs=2, space="PSUM") as pp,
        ):
            w9 = cp.tile([9, 549], f32)
            w72 = cp.tile([ROWS, 81], f32)
            w54 = cp.tile([M2, 216], f32)
            w24 = cp.tile([IH, ROWS], f32)
            eb = cp.tile([ROWS, 3], f32)
            zc = cp.tile([ROWS, 3], f32)
            b9 = cp.tile([9, 1], f32)
            nc.sync.dma_start(w9[:], w9_d)
            nc.sync.dma_start(w72[:], w72_d)
            nc.sync.dma_start(w54[:], w54_d)
            nc.sync.dma_start(w24[:], w24_d)
            nc.sync.dma_start(eb[:], eb_d)
            nc.sync.dma_start(zc[:], zc_d)
            nc.sync.dma_start(b9[:], b9_d)
            # Dummy PE consumers of each const so later matmuls never wait on
            # two DMA-HW queues at once (the LDWEIGHTS struct can hold only
            # one DGE sync-wait).
            warm = pp.tile([1, 8], f32, tag="A")
            for wt in (w9, w72, w54, w24):
                nc.tensor.matmul(warm[:], wt[:, 0:1], wt[:, 0:8],
                                 start=True, stop=True)

            rep_ctx = tc.For_i(0, reps, 1) if reps > 1 else nullcontext()
            with rep_ctx:
              for ci in range(pix // fd):
                o = ci * fd
                aug = sb.tile([9, fd], f32, tag="aug")
                nc.sync.dma_start(aug[:], xall[:, o:o + fd])

                ra_p = pp.tile([M2, fd], f32, tag="A")
                nc.tensor.matmul(ra_p[:], w9[:, W9_RA:W9_RA + M2], aug[:],
                                 start=True, stop=True)
                rb_p = pp.tile([M2, fd], f32, tag="y")
                nc.tensor.matmul(rb_p[:], w9[:, W9_RB:W9_RB + M2], aug[:],
                                 start=True, stop=True)
                rb_s = sb.tile([M2, fd], f32, tag="rb")
                nc.scalar.copy(rb_s[:], rb_p[:])
                m2_s = sb.tile([M2, fd], f32, tag="m2")
                nc.vector.tensor_mul(m2_s[:], ra_p[:], rb_s[:])

                es = []
                for j in range(3):
                    sc_p = pp.tile([ROWS, fd], f32, tag="sc")
                    nc.tensor.matmul(sc_p[:], w54[:, W54_WS[j]:W54_WS[j] + ROWS],
                                     m2_s[:], start=True, stop=False)
                    nc.tensor.matmul(sc_p[:], w9[:, W9_LIN[j]:W9_LIN[j] + ROWS],
                                     aug[:], start=False, stop=True)
                    e_j = sb.tile([ROWS, fd], f32, tag=f"e{j}")
                    nc.scalar.activation(e_j[:], sc_p[:], AF.Exp,
                                         bias=eb[:, j:j + 1])
                    es.append(e_j)

                ps_l = []
                for j in range(3):
                    z_p = pp.tile([ROWS, fd], f32, tag="z")
                    nc.tensor.matmul(z_p[:], w9[:, W9_Z[j]:W9_Z[j] + ROWS], aug[:],
                                     start=True, stop=True)
                    p_j = sb.tile([ROWS, fd], f32, tag=f"p{j}")
                    # p = (z + zc_j) * e   (zc_j is a per-partition scalar)
                    nc.vector.scalar_tensor_tensor(
                        p_j[:], z_p[:], zc[:, j:j + 1], es[j][:],
                        op0=mybir.AluOpType.add, op1=mybir.AluOpType.mult)
                    ps_l.append(p_j)

                T_p = pp.tile([ROWS, fd], f32, tag="A")
                for j in range(3):
                    nc.tensor.matmul(T_p[:], w72[:, W72_I72:W72_I72 + ROWS],
                                     ps_l[j][:], start=(j == 0), stop=(j == 2))
                den_p = pp.tile([ROWS, fd], f32, tag="sc")
                for j in range(3):
                    nc.tensor.matmul(den_p[:], w24[:], es[j][0:IH, :],
                                     start=(j == 0), stop=(j == 2))
                rcp = sb.tile([ROWS, fd], f32, tag="rcp")
                nc.vector.reciprocal(rcp[:], den_p[:])
                msb = sb.tile([ROWS, fd], f32, tag="msb")
                nc.vector.tensor_mul(msb[:], T_p[:], rcp[:])

                out9_p = pp.tile([9, fd], f32, tag="y")
                nc.tensor.matmul(out9_p[:], w72[:, W72_WH:W72_WH + 9], msb[:],
                                 start=True, stop=False)
                # (Ires: residual band add; bias added in the Identity copy)
                nc.tensor.matmul(out9_p[:], w9[:, W9_IRES:W9_IRES + 9], aug[:],
                                 start=False, stop=True)
                res = sb.tile([9, fd], f32, tag="res")
                nc.scalar.activation(res[:], out9_p[:], AF.Identity,
                                     bias=b9[:, 0:1])
                nc.sync.dma_start(oall[:, o:o + fd], res[:])
    nc.compile()
    return nc


def _shard_inputs(band0, band1, band2, wts):
    bands = [np.ascontiguousarray(b, dtype=np.float32)
             for b in (band0, band1, band2)]
    in_maps = []
    for k in range(NCORES):
        b, half = k // 2, k % 2
        m = dict(w9=wts['W9'], w72=wts['W72'], w54=wts['W54'], w24=wts['W24'],
                 eb=wts['EB'], zc=wts['ZC'], b9=wts['BIAS9'])
        m["xall"] = np.concatenate(
            [bands[j][b, :, half * 128:half * 128 + 128, :].reshape(3, PIX)
             for j in range(3)], axis=0)
        in_maps.append(m)
    return in_maps


def _unshard(results):
    full = [np.empty((B, C, H, W), np.float32) for _ in range(3)]
    for k in range(NCORES):
        b, half = k // 2, k % 2
        for j in range(3):
            full[j][b, :, half * 128:half * 128 + 128, :] = \
                results[k]["oall"][3 * j:3 * j + 3].reshape(3, 128, W)
    return tuple(full)


def kernel(band0, band1, band2, Wp, bp, in_proj_w, in_proj_b,
           out_proj_w, out_proj_b, Wo, bo, gates):
    from concourse.bass_utils import run_bass_kernel_spmd

    wts = _merged_weights(np.asarray(Wp), np.asarray(bp), np.asarray(in_proj_w),
                          np.asarray(in_proj_b), np.asarray(out_proj_w),
                          np.asarray(out_proj_b), np.asarray(Wo),
                          np.asarray(bo), np.asarray(gates))
    nc = build_program()
    in_maps = _shard_inputs(np.asarray(band0), np.asarray(band1),
                            np.asarray(band2), wts)
    r = run_bass_kernel_spmd(nc, in_maps, core_ids=list(range(NCORES)))
    return _unshard(r.results)
